# revision 3
# baseline (speedup 1.0000x reference)
"""DKVMN forward kernel for 8 Trainium2 NeuronCores (Bass/Tile).

Strategy (instruction-count-minimal — this environment is dispatch-bound):
 - Data-parallel over batch: core c handles batches [c*32, (c+1)*32).
 - Tables precomputed on device (softmax(k_emb@Mk^T), sigmoid/tanh(v_emb@W^T),
   k_emb@fW2^T+f_b) and stored in DRAM; per-token values fetched by dma_gather.
 - The T=512 recurrence runs as chunked DVE tensor_tensor_scan instructions:
   state m[b,v,k] lives on partitions p=(b_local, k_quarter), free=(k2, v, t).
   One scan instruction advances ALL 1M states by C=4 steps (fp32 internal
   state; segment boundaries handled by a zero-decay slot + carry injection).
 - Reads r[b,t,k] = sum_v w*M_(t-1) via one fused multiply (TT) + one
   segmented tensor_reduce per chunk, using the scan's output trajectory.
 - Final head: f = tanh(fW1@read + KF[item] + f_b), p = sigmoid(p_W.f + p_b)
   with matmuls on PE.
"""
import sys
import numpy as np
import ml_dtypes

sys.path.insert(0, '/opt/trn_rl_repo')

import concourse.bass as bass          # noqa: E402
import concourse.bacc as bacc          # noqa: E402
import concourse.mybir as mybir        # noqa: E402
from concourse.tile import TileContext # noqa: E402
from concourse.bass_utils import run_bass_kernel_spmd  # noqa: E402

F32 = mybir.dt.float32
BF16 = mybir.dt.bfloat16
I16 = mybir.dt.int16
ALU = mybir.AluOpType
ACTF = mybir.ActivationFunctionType
BF = ml_dtypes.bfloat16

NUM_ITEM = 2000
DK = 256          # key dim
DV = 128          # memory slots (v)
B, T = 256, 512
NC = 8
BL = B // NC      # 32 local batches
KSUB = 4          # k quarters on partitions
K2 = DK // KSUB   # 64
P = BL * KSUB     # 128 partitions: p = b*4 + ksub
SEG = K2 * DV     # 8192 cells per partition (k2, v)
C = 4             # scan chunk length (time steps per scan instruction)
SLOT = C + 1      # per-cell slots in D/U (C data + 1 boundary)
NCH = T // C      # 128 chunks
NIT = 2048        # padded item count (16 tiles of 128)
NX = 4096         # padded x count (32 tiles of 128)
TOK = BL * T      # 16384 tokens per core
SCAN_DT = BF16    # D/U/trajectory dtype

_cache = {}
LAST_RESULT = None  # BassKernelResults of the most recent run (for test harness)


def _wrap16(vals):
    """int16 index array [n] -> [128, n/16] wrapped-in-16 + replicated x8."""
    n = len(vals)
    assert n % 16 == 0
    a = np.zeros((16, n // 16), np.int16)
    for i in range(n):
        a[i % 16, i // 16] = vals[i]
    return np.tile(a, (8, 1))


def build_program():
    nc = bacc.Bacc(None, target_bir_lowering=False, debug=False)

    # ---- external inputs (host-prepped) ----
    kT = nc.dram_tensor("kT", [DK, NIT], BF16, kind="ExternalInput")       # k_emb^T padded
    vT = nc.dram_tensor("vT", [DK, NX], BF16, kind="ExternalInput")        # v_emb^T padded
    MkT = nc.dram_tensor("MkT", [DK, DV], BF16, kind="ExternalInput")      # Mk^T
    eaWT = nc.dram_tensor("eaWT", [DK, 2 * DK], BF16, kind="ExternalInput")  # [e_W^T | a_W^T]
    fW2T = nc.dram_tensor("fW2T", [DK, DK], BF16, kind="ExternalInput")    # f_W[:,256:]^T
    fW1T = nc.dram_tensor("fW1T", [DK, DK], F32, kind="ExternalInput")     # f_W[:,:256]^T
    onesf = nc.dram_tensor("onesf", [1, 128], F32, kind="ExternalInput")
    eab = nc.dram_tensor("eab", [1, 2 * DK], F32, kind="ExternalInput")    # [e_b | a_b]
    fb = nc.dram_tensor("fb", [1, DK], F32, kind="ExternalInput")
    pWrep = nc.dram_tensor("pWrep", [128, DK], F32, kind="ExternalInput")  # p_W replicated
    pbcol = nc.dram_tensor("pbcol", [128, 1], F32, kind="ExternalInput")
    m0sh = nc.dram_tensor("m0sh", [P, SEG], SCAN_DT, kind="ExternalInput")     # M0 shifted by one cell
    m0c0 = nc.dram_tensor("m0c0", [P, 1], SCAN_DT, kind="ExternalInput")       # M0 of cell 0
    cidx = nc.dram_tensor("cidx", [P, NCH, 3 * C * P // 16], I16, kind="ExternalInput")
    kfidx = nc.dram_tensor("kfidx", [P, TOK // 16], I16, kind="ExternalInput")

    pred = nc.dram_tensor("pred", [128, TOK // 128], F32, kind="ExternalOutput")

    # ---- DRAM scratch ----
    Wtab = nc.dram_tensor("Wtab", [NIT, DV], F32)            # softmax rows
    Etab = nc.dram_tensor("Etab", [NX * KSUB, K2], F32)      # quarter rows
    Atab = nc.dram_tensor("Atab", [NX * KSUB, K2], F32)
    KFtab = nc.dram_tensor("KFtab", [NIT, DK], F32)
    rT_d = nc.dram_tensor("rT_d", [P, K2, NCH, C], F32)      # reads, scan-native layout
    G1_d = nc.dram_tensor("G1_d", [TOK, DK], BF16)           # fW1@read, token-major

    with TileContext(nc) as tc:
        # ================= stage 1+2: tables =================
        with (
            tc.tile_pool(name="wpool", bufs=1) as wp,
            tc.tile_pool(name="tpool", bufs=1) as tp,
            tc.tile_pool(name="pspool", bufs=2, space="PSUM") as pp,
        ):
            kT_s = [wp.tile([128, NIT], BF16, tag=f"kt{i}", name=f"kt{i}") for i in range(2)]
            vT_s = [wp.tile([128, NX], BF16, tag=f"vt{i}", name=f"vt{i}") for i in range(2)]
            MkT_s = [wp.tile([128, DV], BF16, tag=f"mk{i}", name=f"mk{i}") for i in range(2)]
            eaWT_s = [wp.tile([128, 2 * DK], BF16, tag=f"ea{i}", name=f"eaw{i}") for i in range(2)]
            fW2T_s = [wp.tile([128, DK], BF16, tag=f"f2{i}", name=f"f2{i}") for i in range(2)]
            onesf_s = wp.tile([1, 128], F32, tag="onf")
            eab_s = wp.tile([1, 2 * DK], F32, tag="eb")
            fb_s = wp.tile([1, DK], F32, tag="fb")
            for i in range(2):
                nc.sync.dma_start(kT_s[i][:], kT[128 * i:128 * (i + 1), :])
                nc.sync.dma_start(vT_s[i][:], vT[128 * i:128 * (i + 1), :])
                nc.sync.dma_start(MkT_s[i][:], MkT[128 * i:128 * (i + 1), :])
                nc.sync.dma_start(eaWT_s[i][:], eaWT[128 * i:128 * (i + 1), :])
                nc.sync.dma_start(fW2T_s[i][:], fW2T[128 * i:128 * (i + 1), :])
            nc.sync.dma_start(onesf_s[:], onesf[:])
            nc.sync.dma_start(eab_s[:], eab[:])
            nc.sync.dma_start(fb_s[:], fb[:])

            # --- Wtab: softmax(k_emb @ Mk^T) ---
            wexp = tp.tile([128, 16, DV], F32, tag="wexp")
            for it in range(16):
                ps = pp.tile([128, DV], F32, tag="ps_w")
                sl = slice(128 * it, 128 * (it + 1))
                nc.tensor.matmul(out=ps[:], lhsT=kT_s[0][:, sl], rhs=MkT_s[0][:],
                                 start=True, stop=False)
                nc.tensor.matmul(out=ps[:], lhsT=kT_s[1][:, sl], rhs=MkT_s[1][:],
                                 start=False, stop=True)
                nc.scalar.activation(out=wexp[:, it, :], in_=ps[:], func=ACTF.Exp)
            zs = tp.tile([128, 16], F32, tag="zs")
            nc.vector.tensor_reduce(out=zs[:], in_=wexp[:], axis=mybir.AxisListType.X,
                                    op=ALU.add)
            zr = tp.tile([128, 16], F32, tag="zr")
            nc.vector.reciprocal(out=zr[:], in_=zs[:])
            nc.vector.tensor_tensor(
                out=wexp[:], in0=wexp[:],
                in1=zr[:].unsqueeze(2).to_broadcast([128, 16, DV]), op=ALU.mult)
            # DRAM write: row (it*128+p) -> iterate [p, it, v]
            nc.sync.dma_start(
                Wtab[:].rearrange("(it p) v -> p it v", p=128), wexp[:])

            # --- Etab/Atab: sigmoid/tanh(v_emb @ [eW|aW]^T + [eb|ab]) ---
            ea = tp.tile([128, 32, 2 * DK], F32, tag="ea")
            for it in range(32):
                ps = pp.tile([128, 2 * DK], F32, tag="ps_ea")
                sl = slice(128 * it, 128 * (it + 1))
                nc.tensor.matmul(out=ps[:], lhsT=vT_s[0][:, sl], rhs=eaWT_s[0][:],
                                 start=True, stop=False)
                nc.tensor.matmul(out=ps[:], lhsT=vT_s[1][:, sl], rhs=eaWT_s[1][:],
                                 start=False, stop=False)
                nc.tensor.matmul(out=ps[:], lhsT=onesf_s[:], rhs=eab_s[:],
                                 start=False, stop=True)
                nc.scalar.activation(out=ea[:, it, 0:DK], in_=ps[:, 0:DK], func=ACTF.Sigmoid)
                nc.scalar.activation(out=ea[:, it, DK:2 * DK], in_=ps[:, DK:2 * DK],
                                     func=ACTF.Tanh)
            # quarter-row layout: row (x*4+q) = ea[p, it, table, q*64:(q+1)*64], x = it*128+p
            nc.sync.dma_start(
                Etab[:].rearrange("(it p q) c -> p it q c", p=128, q=KSUB),
                ea[:, :, 0:DK].rearrange("p it (q c) -> p it q c", q=KSUB))
            nc.sync.dma_start(
                Atab[:].rearrange("(it p q) c -> p it q c", p=128, q=KSUB),
                ea[:, :, DK:2 * DK].rearrange("p it (q c) -> p it q c", q=KSUB))

            # --- KFtab: k_emb @ fW2^T + f_b ---
            kf = tp.tile([128, 16, DK], F32, tag="kf")
            for it in range(16):
                ps = pp.tile([128, DK], F32, tag="ps_kf")
                sl = slice(128 * it, 128 * (it + 1))
                nc.tensor.matmul(out=ps[:], lhsT=kT_s[0][:, sl], rhs=fW2T_s[0][:],
                                 start=True, stop=False)
                nc.tensor.matmul(out=ps[:], lhsT=kT_s[1][:, sl], rhs=fW2T_s[1][:],
                                 start=False, stop=False)
                nc.tensor.matmul(out=ps[:], lhsT=onesf_s[:], rhs=fb_s[:],
                                 start=False, stop=True)
                nc.scalar.copy(out=kf[:, it, :], in_=ps[:])
            nc.sync.dma_start(
                KFtab[:].rearrange("(it p) c -> p it c", p=128), kf[:])

        # ================= stage 3: the scan =================
        with (
            tc.tile_pool(name="scst", bufs=1) as st,
            tc.tile_pool(name="scg", bufs=2) as sg,
        ):
            Dt = st.tile([P, SEG * SLOT], SCAN_DT, tag="D")
            Ut = st.tile([P, 1 + SEG * SLOT], SCAN_DT, tag="U")
            # D boundary slots (flat j*SLOT + C) = 0, once
            nc.vector.memset(Dt[:].rearrange("p (s j) -> p s j", j=SLOT)[:, :, C:], 0.0)
            # U init: pad col = M0(cell0); slot-C of cell s = M0(cell s+1)
            nc.sync.dma_start(Ut[:, 0:1], m0c0[:])
            m0st = st.tile([P, SEG], SCAN_DT, tag="m0st")
            nc.sync.dma_start(m0st[:], m0sh[:])
            nc.vector.tensor_copy(
                out=Ut[:, 1:].rearrange("p (s j) -> p s j", j=SLOT)[:, :, C:].squeeze(2),
                in_=m0st[:])

            rT_sb = st.tile([P, K2, C], F32, tag="rt")

            d5 = Dt[:].rearrange("p (k v j) -> p k v j", k=K2, j=SLOT)
            u_data = Ut[:, 1:].rearrange("p (k v j) -> p k v j", k=K2, j=SLOT)

            for ch in range(NCH):
                ix = sg.tile([P, 3 * C * P // 16], I16, tag="ix")
                nc.sync.dma_start(ix[:], cidx[:, ch, :])
                w_g = sg.tile([P, C, DV], F32, tag="wg")
                e_g = sg.tile([P, C, K2], F32, tag="eg")
                a_g = sg.tile([P, C, K2], F32, tag="ag")
                nw = C * P // 16
                nc.gpsimd.dma_gather(w_g[:], Wtab[:], ix[:, 0:nw], C * P, C * P, DV)
                nc.gpsimd.dma_gather(e_g[:], Etab[:], ix[:, nw:2 * nw], C * P, C * P, K2)
                nc.gpsimd.dma_gather(a_g[:], Atab[:], ix[:, 2 * nw:3 * nw], C * P, C * P, K2)

                # broadcast access patterns over (k2, v, t)
                w_ap = w_g[:].rearrange("p t v -> p t v").unsqueeze(1) \
                    .to_broadcast([P, K2, C, DV]).transpose([0, 1, 3, 2])
                e_ap = e_g[:].rearrange("p t k -> p t k").unsqueeze(2) \
                    .to_broadcast([P, C, DV, K2]).transpose([0, 3, 2, 1])

                # V = w*e -> D[.., 0:C]; then D = 1 - V
                nc.vector.tensor_tensor(out=d5[:, :, :, 0:C], in0=w_ap, in1=e_ap,
                                        op=ALU.mult)
                nc.vector.tensor_scalar(out=d5[:, :, :, 0:C], in0=d5[:, :, :, 0:C],
                                        scalar1=-1.0, scalar2=1.0,
                                        op0=ALU.mult, op1=ALU.add)
                # U[.., 0:C] = w*a
                a_ap = a_g[:].rearrange("p t k -> p t k").unsqueeze(2) \
                    .to_broadcast([P, C, DV, K2]).transpose([0, 3, 2, 1])
                nc.vector.tensor_tensor(out=u_data[:, :, :, 0:C], in0=w_ap, in1=a_ap,
                                        op=ALU.mult)
                # scan (out aliases U data region); initial = pad column
                nc.vector.tensor_tensor_scan(
                    out=Ut[:, 1:], data0=Dt[:], data1=Ut[:, 1:],
                    initial=Ut[:, 0:1], op0=ALU.mult, op1=ALU.add)
                # reads: P = M_(t-1) * w -> D[.., 0:C]; M_(t-1)(s) at flat 5s+t-1
                mprev = Ut[:, 0:SEG * SLOT].rearrange(
                    "p (k v j) -> p k v j", k=K2, j=SLOT)[:, :, :, 0:C]
                nc.vector.tensor_tensor(out=d5[:, :, :, 0:C], in0=mprev, in1=w_ap,
                                        op=ALU.mult)
                nc.vector.tensor_reduce(
                    out=rT_sb[:], in_=d5[:, :, :, 0:C].transpose([0, 1, 3, 2]),
                    axis=mybir.AxisListType.X, op=ALU.add)
                nc.sync.dma_start(rT_d[:, :, ch, :], rT_sb[:])
                if ch + 1 < NCH:
                    # carry: pad <- end-state(cell0); slot-C(s) <- end-state(s+1)
                    nc.vector.tensor_copy(out=Ut[:, 0:1], in_=Ut[:, C:C + 1])
                    nc.vector.tensor_copy(
                        out=Ut[:, 1:].rearrange("p (s j) -> p s j", j=SLOT)[:, 0:SEG - 1, C:],
                        in_=Ut[:, 1:].rearrange("p (s j) -> p s j", j=SLOT)[:, 1:SEG, C - 1:C])

        # ================= stage 4: head =================
        with (
            tc.tile_pool(name="hw", bufs=1) as hw,
            tc.tile_pool(name="hp", bufs=1) as hpool,
            tc.tile_pool(name="hps", bufs=4, space="PSUM") as hps,
        ):
            fW1_s = [hw.tile([128, DK], F32, tag=f"f1{i}", name=f"f1{i}") for i in range(2)]
            for i in range(2):
                nc.sync.dma_start(fW1_s[i][:], fW1T[128 * i:128 * (i + 1), :])
            # G1 = fW1 @ read : process tokens in quarters
            QT = TOK // 4  # 4096 tokens
            for q in range(4):
                rq = [hpool.tile([128, QT], F32, tag=f"rq{h}", name=f"rq{h}") for h in range(2)]
                for h in range(2):
                    for j in range(2):
                        ks = 2 * h + j
                        src = rT_d[:].rearrange(
                            "(b ks) k ch t -> ks k b ch t", ks=KSUB)[
                            ks, :, q * 8:(q + 1) * 8, :, :]
                        nc.sync.dma_start(rq[h][64 * j:64 * (j + 1), :], src)
                g1 = hpool.tile([128, 2, QT], BF16, tag="g1")
                for m in range(2):
                    for n in range(QT // 512):
                        ps = hps.tile([128, 512], F32, tag="psh")
                        nsl = slice(512 * n, 512 * (n + 1))
                        nc.tensor.matmul(out=ps[:], lhsT=fW1_s[0][:, 128 * m:128 * (m + 1)],
                                         rhs=rq[0][:, nsl], start=True, stop=False)
                        nc.tensor.matmul(out=ps[:], lhsT=fW1_s[1][:, 128 * m:128 * (m + 1)],
                                         rhs=rq[1][:, nsl], start=False, stop=True)
                        nc.scalar.copy(out=g1[:, m, nsl], in_=ps[:])
                # token-major DRAM: token tok0 = q*4096 + j ; G1_d[tok, k]
                for m in range(2):
                    nc.sync.dma_start(
                        G1_d[q * QT:(q + 1) * QT, 128 * m:128 * (m + 1)]
                        .rearrange("j k -> k j"), g1[:, m, :])

            # f = tanh(G1 + KF), pred = sigmoid(p.f + pb)
            pW_s = hw.tile([128, DK], F32, tag="pw")
            pb_s = hw.tile([128, 1], F32, tag="pb")
            kfi_s = hw.tile([P, TOK // 16], I16, tag="kfi")
            nc.sync.dma_start(pW_s[:], pWrep[:])
            nc.sync.dma_start(pb_s[:], pbcol[:])
            nc.sync.dma_start(kfi_s[:], kfidx[:])
            prow = hw.tile([128, TOK // 128], F32, tag="prow")
            for q in range(4):
                # tokens tok = q*4096 + blk*128 + p, blk in [0,32)
                g1q = hpool.tile([128, 32, DK], BF16, tag="g1q")
                nc.sync.dma_start(
                    g1q[:], G1_d[q * QT:(q + 1) * QT, :].rearrange("(blk p) k -> p blk k", p=128))
                kfg = hpool.tile([128, 32, DK], F32, tag="kfg")
                for g in range(4):
                    nc.gpsimd.dma_gather(
                        kfg[:, 8 * g:8 * (g + 1), :], KFtab[:],
                        kfi_s[:, (q * 4 + g) * 64:(q * 4 + g + 1) * 64],
                        1024, 1024, DK)
                fq = hpool.tile([128, 32, DK], BF16, tag="fq")
                nc.vector.tensor_tensor(out=fq[:], in0=g1q[:], in1=kfg[:], op=ALU.add)
                nc.scalar.activation(out=fq[:], in_=fq[:], func=ACTF.Tanh)
                nc.vector.tensor_tensor(
                    out=fq[:], in0=fq[:],
                    in1=pW_s[:].unsqueeze(1).to_broadcast([128, 32, DK]), op=ALU.mult)
                nc.vector.tensor_reduce(out=prow[:, 32 * q:32 * (q + 1)], in_=fq[:],
                                        axis=mybir.AxisListType.X, op=ALU.add)
            nc.scalar.activation(out=prow[:], in_=prow[:], func=ACTF.Sigmoid,
                                 bias=pb_s[:])
            nc.sync.dma_start(pred[:], prow[:])

    nc.finalize()
    return nc


def _host_prep(item_seq, correct_seq, k_emb, v_emb, Mk, Mv0, e_W, e_b, a_W, a_b,
               f_W, f_b, p_W, p_b):
    """Shared (core-independent) input prep."""
    pad_k = np.zeros((NIT, DK), np.float32)
    pad_k[:NUM_ITEM] = k_emb
    pad_v = np.zeros((NX, DK), np.float32)
    pad_v[:2 * NUM_ITEM] = v_emb
    shared = {
        "kT": np.ascontiguousarray(pad_k.T).astype(BF),
        "vT": np.ascontiguousarray(pad_v.T).astype(BF),
        "MkT": np.ascontiguousarray(Mk.T).astype(BF),
        "eaWT": np.ascontiguousarray(np.concatenate([e_W.T, a_W.T], axis=1)).astype(BF),
        "fW2T": np.ascontiguousarray(f_W[:, DK:].T).astype(BF),
        "fW1T": np.ascontiguousarray(f_W[:, :DK].T).astype(np.float32),
        "onesf": np.ones((1, 128), np.float32),
        "eab": np.concatenate([e_b, a_b])[None, :].astype(np.float32),
        "fb": f_b[None, :].astype(np.float32),
        "pWrep": np.tile(p_W.reshape(1, DK), (128, 1)).astype(np.float32),
        "pbcol": np.full((128, 1), float(p_b[0]), np.float32),
    }
    # M0 in cell layout: cell s=(k2, v); partition p=(b, ksub)
    # M0[p, s] = Mv0[v, ksub*64+k2]
    ks = np.arange(P) % KSUB                       # [P]
    k2i, vi = np.meshgrid(np.arange(K2), np.arange(DV), indexing="ij")
    m0_cell = Mv0.T[(ks[:, None, None] * K2 + k2i[None]), vi[None]]  # [P, K2, DV]
    m0_flat = m0_cell.reshape(P, SEG).astype(np.float32)
    m0sh = np.zeros((P, SEG), np.float32)
    m0sh[:, :SEG - 1] = m0_flat[:, 1:]
    np_scan_dt = BF if SCAN_DT == BF16 else np.float32
    shared["m0sh"] = m0sh.astype(np_scan_dt)
    shared["m0c0"] = m0_flat[:, 0:1].astype(np_scan_dt)
    return shared


def _core_idx(item_c, x_c):
    """Per-core gather index tensors. item_c/x_c: [BL, T] int arrays."""
    bl = np.arange(P) // KSUB
    ks = np.arange(P) % KSUB
    nw = C * P // 16
    cidx = np.zeros((P, NCH, 3 * nw), np.int16)
    for ch in range(NCH):
        tt = ch * C + np.arange(C)
        # vector i = t_local*128 + p
        witem = item_c[bl[None, :], tt[:, None]].reshape(-1)          # [C*P]
        xq = (x_c[bl[None, :], tt[:, None]] * KSUB + ks[None, :]).reshape(-1)
        cidx[:, ch, 0:nw] = _wrap16(witem.astype(np.int64))
        cidx[:, ch, nw:2 * nw] = _wrap16(xq.astype(np.int64))
        cidx[:, ch, 2 * nw:3 * nw] = _wrap16(xq.astype(np.int64))
    # kf: token = b*512 + t ; vector i = tok
    kf_items = item_c.reshape(-1)
    kfidx = _wrap16(kf_items.astype(np.int64))
    return {"cidx": cidx, "kfidx": kfidx}


def kernel(**inputs):
    inputs = {k: np.asarray(v) for k, v in inputs.items()}
    item = inputs["item_seq"].astype(np.int64)
    corr = inputs["correct_seq"].astype(np.int64)
    x = item + NUM_ITEM * corr

    if "nc" not in _cache:
        _cache["nc"] = build_program()
    nc = _cache["nc"]

    shared = _host_prep(
        item, corr,
        inputs["k_emb"].astype(np.float32), inputs["v_emb"].astype(np.float32),
        inputs["Mk"].astype(np.float32), inputs["Mv0"].astype(np.float32),
        inputs["e_W"].astype(np.float32), inputs["e_b"].astype(np.float32),
        inputs["a_W"].astype(np.float32), inputs["a_b"].astype(np.float32),
        inputs["f_W"].astype(np.float32), inputs["f_b"].astype(np.float32),
        inputs["p_W"].astype(np.float32), inputs["p_b"].astype(np.float32))

    in_maps = []
    for c in range(NC):
        sl = slice(c * BL, (c + 1) * BL)
        m = dict(shared)
        m.update(_core_idx(item[sl], x[sl]))
        in_maps.append(m)

    import os
    tdir = os.environ.get("BASS_KERNEL_TRACE_DIR")
    res = run_bass_kernel_spmd(nc, in_maps, core_ids=list(range(NC)),
                               tmpdir=tdir if tdir else None)
    global LAST_RESULT
    LAST_RESULT = res

    out = np.zeros((B, T), np.float32)
    blk = np.arange(TOK // 128)
    pp_, bb_ = np.meshgrid(np.arange(128), blk, indexing="ij")
    tok = bb_ * 128 + pp_          # token id at [p, blk]
    for c in range(NC):
        pr = res.results[c]["pred"]          # [128, TOK//128]
        b_l, t_l = tok // T, tok % T
        out[c * BL + b_l, t_l] = pr
    return out


if __name__ == "__main__":
    # smoke test vs numpy reference
    import time
    rng = np.random.default_rng(0)
    s = 0.05
    ins = {
        "item_seq": rng.integers(0, NUM_ITEM, (B, T)),
        "correct_seq": rng.integers(0, 2, (B, T)),
        "k_emb": (rng.standard_normal((NUM_ITEM, DK)) * s).astype(np.float32),
        "v_emb": (rng.standard_normal((2 * NUM_ITEM, DK)) * s).astype(np.float32),
        "Mk": (rng.standard_normal((DV, DK)) * s).astype(np.float32),
        "Mv0": (rng.standard_normal((DV, DK)) * s).astype(np.float32),
        "e_W": (rng.standard_normal((DK, DK)) * s).astype(np.float32),
        "e_b": np.zeros(DK, np.float32),
        "a_W": (rng.standard_normal((DK, DK)) * s).astype(np.float32),
        "a_b": np.zeros(DK, np.float32),
        "f_W": (rng.standard_normal((DK, 2 * DK)) * s).astype(np.float32),
        "f_b": np.zeros(DK, np.float32),
        "p_W": (rng.standard_normal((1, DK)) * s).astype(np.float32),
        "p_b": np.zeros(1, np.float32),
    }
    t0 = time.time()
    out = kernel(**ins)
    print("kernel wall:", time.time() - t0)

    # numpy reference
    k = ins["k_emb"][ins["item_seq"]]
    v = ins["v_emb"][ins["item_seq"] + NUM_ITEM * ins["correct_seq"]]
    logits = k @ ins["Mk"].T
    w = np.exp(logits - logits.max(-1, keepdims=True))
    w /= w.sum(-1, keepdims=True)
    e = 1 / (1 + np.exp(-(v @ ins["e_W"].T + ins["e_b"])))
    a = np.tanh(v @ ins["a_W"].T + ins["a_b"])
    M = np.broadcast_to(ins["Mv0"][None], (B, DV, DK)).copy()
    reads = np.zeros((B, T, DK), np.float32)
    for t in range(T):
        reads[:, t] = np.einsum("bv,bvk->bk", w[:, t], M)
        M = M * (1 - w[:, t][:, :, None] * e[:, t][:, None, :]) \
            + w[:, t][:, :, None] * a[:, t][:, None, :]
    f = np.tanh(np.concatenate([reads, k], -1) @ ins["f_W"].T + ins["f_b"])
    ref = 1 / (1 + np.exp(-(f @ ins["p_W"].T + ins["p_b"])))[:, :, 0]
    err = np.abs(out - ref)
    print("max abs err:", err.max(), " rel:", err.max() / np.abs(ref).max())



# revision 24
# speedup vs baseline: 3.7586x; 3.7586x over previous
"""DKVMN forward kernel for 8 Trainium2 NeuronCores (Bass/Tile) — v2.

Design (replaces the tensor_tensor_scan/slot design):
 - Data-parallel over batch: core c handles batches [c*32, (c+1)*32).
 - State layout: p = b_local*4 + kq (128 partitions), free = (k2=64, v=128);
   M[p, k2*128+v] = M_state[b][v, kq*64+k2], bf16.
 - Device-built DRAM table DUtab[(x, tab, q)] of quarter-rows (8192 core
   elems + 128 tail): D = 1 - w[item(x)] (x) e[x] (outer, k-major/v-inner),
   U = w (x) a.  D-rows carry w[item] in the tail for the read-multiply.
 - Per chunk (C=2 steps): two dma_gathers (16.6KB rows, ~8.5MB/chunk) fetch
   D/U/w for 2 steps; per step 4 packed DVE ops (all 2x-mode eligible):
     RT = M * w_bcast ; r = reduce_v(RT) ; M = M * D ; M = M + U
 - reads stored bf16 token-major [tok, k]; head uses dma_start_transpose
   (xbar) to get [k, tok] tiles, PE matmuls for fW1@r and p_W.f, KF via
   transposed dma_gather.  No strided 4-byte DMA anywhere.
"""
import os
import sys
import numpy as np
import ml_dtypes

sys.path.insert(0, '/opt/trn_rl_repo')

import concourse.bass as bass          # noqa: E402
import concourse.bacc as bacc          # noqa: E402
import concourse.mybir as mybir        # noqa: E402
from concourse.tile import TileContext # noqa: E402
from concourse.bass_utils import run_bass_kernel_spmd  # noqa: E402

F32 = mybir.dt.float32
BF16 = mybir.dt.bfloat16
I16 = mybir.dt.int16
ALU = mybir.AluOpType
ACTF = mybir.ActivationFunctionType
BF = ml_dtypes.bfloat16

NUM_ITEM = 2000
DK = 256          # key dim
DV = 128          # memory slots (v)
B, T = 256, 512
NC = 8
BL = B // NC      # 32 local batches
KSUB = 4          # k quarters on partitions
K2 = DK // KSUB   # 64
P = BL * KSUB     # 128 partitions: p = kq*32 + b (kq-major)
SEG = K2 * DV     # 8192 state cells per partition
C = 2             # steps per chunk
NCH = T // C      # 256 chunks
NIT = 2048        # padded item count
NX = 4096         # padded x count (e/a tables)
NXR = 4000        # real x count (D/U tables)
ROW = SEG + DV    # 8320 elems per D quarter-row (core + w tail)
TOK = BL * T      # 16384 tokens per core
TH = TOK // 2     # head token-half

_cache = {}
LAST_RESULT = None  # BassKernelResults of the most recent run (for test harness)


def _wrap16(vals):
    """int index array [n] -> [128, n/16] wrapped-in-16 + replicated x8."""
    vals = np.asarray(vals, np.int64)
    n = len(vals)
    assert n % 16 == 0
    a = np.zeros((16, n // 16), np.int16)
    a[np.arange(n) % 16, np.arange(n) // 16] = vals
    return np.tile(a, (8, 1))


def build_program():
    nc = bacc.Bacc(None, target_bir_lowering=False, debug=False)

    # ---- external inputs (host-prepped) ----
    kT = nc.dram_tensor("kT", [DK, NIT], BF16, kind="ExternalInput")        # k_emb^T padded
    vT = nc.dram_tensor("vT", [DK, NX], BF16, kind="ExternalInput")         # v_emb^T padded
    MkT = nc.dram_tensor("MkT", [DK, DV], BF16, kind="ExternalInput")       # Mk^T
    eaWT = nc.dram_tensor("eaWT", [DK, 2 * DK], BF16, kind="ExternalInput")  # [e_W^T | a_W^T]
    fW2T = nc.dram_tensor("fW2T", [DK, DK], BF16, kind="ExternalInput")     # f_W[:,256:]^T
    fW1T = nc.dram_tensor("fW1T", [DK, DK], BF16, kind="ExternalInput")     # f_W[:,:256]^T
    onesf = nc.dram_tensor("onesf", [1, 128], F32, kind="ExternalInput")
    eab = nc.dram_tensor("eab", [1, 2 * DK], F32, kind="ExternalInput")     # [e_b | a_b]
    fbcol = nc.dram_tensor("fbcol", [128, 2], F32, kind="ExternalInput")    # f_b by ko-half
    pwcol = nc.dram_tensor("pwcol", [128, 2], BF16, kind="ExternalInput")   # p_W by ko-half
    pbt = nc.dram_tensor("pbt", [1, 1], F32, kind="ExternalInput")
    m0 = nc.dram_tensor("m0", [P, SEG], BF16, kind="ExternalInput")
    cidx = nc.dram_tensor("cidx", [P, NCH * 16], I16, kind="ExternalInput")
    wxidx = nc.dram_tensor("wxidx", [P, NX // 16], I16, kind="ExternalInput")
    kfidx = nc.dram_tensor("kfidx", [P, TOK // 16], I16, kind="ExternalInput")

    pred = nc.dram_tensor("pred", [1, TOK], F32, kind="ExternalOutput")

    # ---- DRAM scratch ----
    WtabD = nc.dram_tensor("WtabD", [NIT, DV], BF16)      # softmax rows (item)
    KFtabD = nc.dram_tensor("KFtabD", [NIT, DK], BF16)    # k_emb @ fW2^T rows
    Dtab = nc.dram_tensor("Dtab", [NXR * 4, ROW], BF16)   # (x, q): 1-w(x)e + w tail
    Utab = nc.dram_tensor("Utab", [NXR * 4, SEG], BF16)   # (x, q): w(x)a
    rT_d2 = nc.dram_tensor("rT_d2", [TOK, DK], BF16)      # reads, token-major

    with TileContext(nc) as tc:
        # ================= stage 1a: w / e / a / kf tables =================
        with tc.tile_pool(name="keep", bufs=1) as kp:
            ea_bf = kp.tile([128, 32, 2 * DK], BF16, tag="eabf")   # sig/tanh rows (x)
            wx_sb = kp.tile([128, 32, DV], BF16, tag="wx")         # w rows by x
            with (
                tc.tile_pool(name="wpool", bufs=1) as wp,
                tc.tile_pool(name="tpool", bufs=1) as tp,
                tc.tile_pool(name="pspool", bufs=2, space="PSUM") as pp,
            ):
                kT_s = [wp.tile([128, NIT], BF16, tag=f"kt{i}", name=f"kt{i}") for i in range(2)]
                vT_s = [wp.tile([128, NX], BF16, tag=f"vt{i}", name=f"vt{i}") for i in range(2)]
                MkT_s = [wp.tile([128, DV], BF16, tag=f"mk{i}", name=f"mk{i}") for i in range(2)]
                eaWT_s = [wp.tile([128, 2 * DK], BF16, tag=f"ea{i}", name=f"eaw{i}") for i in range(2)]
                fW2T_s = [wp.tile([128, DK], BF16, tag=f"f2{i}", name=f"f2{i}") for i in range(2)]
                onesf_s = wp.tile([1, 128], F32, tag="onf")
                eab_s = wp.tile([1, 2 * DK], F32, tag="eb")
                for i in range(2):
                    nc.sync.dma_start(kT_s[i][:], kT[128 * i:128 * (i + 1), :])
                    nc.sync.dma_start(vT_s[i][:], vT[128 * i:128 * (i + 1), :])
                    nc.sync.dma_start(MkT_s[i][:], MkT[128 * i:128 * (i + 1), :])
                    nc.sync.dma_start(eaWT_s[i][:], eaWT[128 * i:128 * (i + 1), :])
                    nc.sync.dma_start(fW2T_s[i][:], fW2T[128 * i:128 * (i + 1), :])
                nc.sync.dma_start(onesf_s[:], onesf[:])
                nc.sync.dma_start(eab_s[:], eab[:])

                # --- softmax(k_emb @ Mk^T) rows -> WtabD (bf16) ---
                wexp = tp.tile([128, 16, DV], F32, tag="wexp")
                for it in range(16):
                    ps = pp.tile([128, DV], F32, tag="ps_w")
                    sl = slice(128 * it, 128 * (it + 1))
                    nc.tensor.matmul(out=ps[:], lhsT=kT_s[0][:, sl], rhs=MkT_s[0][:],
                                     start=True, stop=False)
                    nc.tensor.matmul(out=ps[:], lhsT=kT_s[1][:, sl], rhs=MkT_s[1][:],
                                     start=False, stop=True)
                    nc.scalar.activation(out=wexp[:, it, :], in_=ps[:], func=ACTF.Exp)
                zs = tp.tile([128, 16], F32, tag="zs")
                nc.vector.tensor_reduce(out=zs[:], in_=wexp[:], axis=mybir.AxisListType.X,
                                        op=ALU.add)
                zr = tp.tile([128, 16], F32, tag="zr")
                nc.vector.reciprocal(out=zr[:], in_=zs[:])
                wbf = tp.tile([128, 16, DV], BF16, tag="wbf")
                nc.vector.tensor_tensor(
                    out=wbf[:], in0=wexp[:],
                    in1=zr[:].unsqueeze(2).to_broadcast([128, 16, DV]), op=ALU.mult)
                nc.sync.dma_start(
                    WtabD[:].rearrange("(it p) v -> p it v", p=128), wbf[:])

                # --- sigmoid/tanh(v_emb @ [eW|aW]^T + [eb|ab]) rows (by x) ---
                for it in range(32):
                    ps = pp.tile([128, 2 * DK], F32, tag="ps_ea")
                    sl = slice(128 * it, 128 * (it + 1))
                    nc.tensor.matmul(out=ps[:], lhsT=vT_s[0][:, sl], rhs=eaWT_s[0][:],
                                     start=True, stop=False)
                    nc.tensor.matmul(out=ps[:], lhsT=vT_s[1][:, sl], rhs=eaWT_s[1][:],
                                     start=False, stop=False)
                    nc.tensor.matmul(out=ps[:], lhsT=onesf_s[:], rhs=eab_s[:],
                                     start=False, stop=True)
                    nc.scalar.activation(out=ea_bf[:, it, 0:DK], in_=ps[:, 0:DK],
                                         func=ACTF.Sigmoid)
                    nc.scalar.activation(out=ea_bf[:, it, DK:2 * DK], in_=ps[:, DK:2 * DK],
                                         func=ACTF.Tanh)

                # --- KFtabD: k_emb @ fW2^T (NO f_b; added later via ACT bias) ---
                kfbf = tp.tile([128, 16, DK], BF16, tag="kfbf")
                for it in range(16):
                    ps = pp.tile([128, DK], F32, tag="ps_kf")
                    sl = slice(128 * it, 128 * (it + 1))
                    nc.tensor.matmul(out=ps[:], lhsT=kT_s[0][:, sl], rhs=fW2T_s[0][:],
                                     start=True, stop=False)
                    nc.tensor.matmul(out=ps[:], lhsT=kT_s[1][:, sl], rhs=fW2T_s[1][:],
                                     start=False, stop=True)
                    nc.scalar.copy(out=kfbf[:, it, :], in_=ps[:])
                nc.sync.dma_start(
                    KFtabD[:].rearrange("(it p) c -> p it c", p=128), kfbf[:])

                # w rows re-fetched x-aligned (item(x) = x mod 2000)
                wxi_s = tp.tile([P, NX // 16], I16, tag="wxi")
                nc.sync.dma_start(wxi_s[:], wxidx[:])
                for gq in range(4):
                    nc.gpsimd.dma_gather(
                        wx_sb[:, 8 * gq:8 * (gq + 1), :], WtabD[:],
                        wxi_s[:, 64 * gq:64 * (gq + 1)], NX // 4, NX // 4, DV)

            # ================= stage 1b: D/U table build =================
            D4 = Dtab[:].rearrange("(x q) e -> x (q e)", q=KSUB)
            U4 = Utab[:].rearrange("(x q) e -> x q e", q=KSUB)
            with tc.tile_pool(name="bpool", bufs=2) as bp:
                for g in range(32):
                    npart = min(128, NXR - 128 * g)   # last tile covers 32 x's
                    xsl = slice(128 * g, 128 * g + npart)
                    for tab in range(2):
                        BT = bp.tile([128, KSUB, ROW], BF16, tag="bt")
                        core = BT[:, :, 0:SEG].rearrange(
                            "p q (k2 v) -> p q k2 v", v=DV)      # [p, 4, 64, 128]
                        col = slice(0, DK) if tab == 0 else slice(DK, 2 * DK)
                        e4 = ea_bf[:, g, col].rearrange(
                            "p (q k2) -> p q k2", q=KSUB).unsqueeze(3) \
                            .to_broadcast([128, KSUB, K2, DV])
                        w4 = wx_sb[:, g, :].unsqueeze(1).unsqueeze(2) \
                            .to_broadcast([128, KSUB, K2, DV])
                        nc.vector.tensor_tensor(out=core, in0=e4, in1=w4, op=ALU.mult)
                        if tab == 0:
                            nc.vector.tensor_scalar(
                                out=core, in0=core, scalar1=-1.0, scalar2=1.0,
                                op0=ALU.mult, op1=ALU.add)
                            nc.vector.tensor_copy(
                                out=BT[:, :, SEG:ROW],
                                in_=wx_sb[:, g, :].unsqueeze(1).to_broadcast(
                                    [128, KSUB, DV]))
                            nc.sync.dma_start(
                                D4[xsl, :],
                                BT[0:npart].rearrange("p q e -> p (q e)"))
                        else:
                            nc.sync.dma_start(
                                U4[xsl], BT[0:npart, :, 0:SEG])

        # ================= stage 2: recurrence =================
        with (
            tc.tile_pool(name="scst", bufs=1) as st,
            tc.tile_pool(name="scg", bufs=2) as sg,
            tc.tile_pool(name="rp", bufs=2) as rp,
        ):
            Mt = st.tile([P, SEG], BF16, tag="M")
            RT = st.tile([P, SEG], BF16, tag="RT")
            cidx_sb = st.tile([P, NCH, 16], I16, tag="cix")
            nc.sync.dma_start(Mt[:], m0[:])
            nc.sync.dma_start(cidx_sb[:].rearrange("p a b -> p (a b)"), cidx[:])

            M3 = Mt[:].rearrange("p (k v) -> p k v", v=DV)
            RT3 = RT[:].rearrange("p (k v) -> p k v", v=DV)
            R2 = rT_d2[:].rearrange("(b t) k -> b t k", b=BL)

            for ch in range(NCH):
                Dg = sg.tile([P, C, ROW], BF16, tag="dg")
                Ug = sg.tile([P, C, SEG], BF16, tag="ug")
                nc.gpsimd.dma_gather(Dg[:], Dtab[:],
                                     cidx_sb[:, ch, :], C * P, C * P, ROW)
                nc.gpsimd.dma_gather(Ug[:], Utab[:],
                                     cidx_sb[:, ch, :], C * P, C * P, SEG)
                rTf = rp.tile([P, C, K2], F32, tag="rtf")
                rTb = rp.tile([P, C, K2], BF16, tag="rtb")
                for s in range(C):
                    w_ap = Dg[:, s, SEG:ROW].unsqueeze(1).to_broadcast([P, K2, DV])
                    nc.vector.tensor_tensor(out=RT3, in0=M3, in1=w_ap, op=ALU.mult)
                    nc.vector.tensor_reduce(out=rTf[:, s, :], in_=RT3,
                                            axis=mybir.AxisListType.X, op=ALU.add)
                    nc.vector.tensor_tensor(out=Mt[:], in0=Mt[:],
                                            in1=Dg[:, s, 0:SEG], op=ALU.mult)
                    nc.vector.tensor_tensor(out=Mt[:], in0=Mt[:],
                                            in1=Ug[:, s, :], op=ALU.add)
                nc.scalar.copy(out=rTb[:], in_=rTf[:])
                for kq in range(KSUB):
                    nc.sync.dma_start(
                        R2[:, C * ch:C * (ch + 1), K2 * kq:K2 * (kq + 1)],
                        rTb[BL * kq:BL * (kq + 1), :, :])

        # ================= stage 3: head =================
        with (
            tc.tile_pool(name="hw", bufs=1) as hw,
            tc.tile_pool(name="hp", bufs=1) as hpool,
            tc.tile_pool(name="hps", bufs=4, space="PSUM") as hps,
            tc.tile_pool(name="hps2", bufs=4, space="PSUM") as hps2,
        ):
            f1 = [hw.tile([128, DK], BF16, tag=f"f1{h}", name=f"f1{h}") for h in range(2)]
            for h in range(2):
                nc.sync.dma_start(f1[h][:], fW1T[128 * h:128 * (h + 1), :])
            pw_s = hw.tile([128, 2], BF16, tag="pw")
            fb_s = hw.tile([128, 2], F32, tag="fb")
            pb_s = hw.tile([1, 1], F32, tag="pb")
            kfi_s = hw.tile([P, TOK // 16], I16, tag="kfi")
            pred_sb = hw.tile([1, TOK], F32, tag="prd")
            nc.sync.dma_start(pw_s[:], pwcol[:])
            nc.sync.dma_start(fb_s[:], fbcol[:])
            nc.sync.dma_start(pb_s[:], pbt[:])
            nc.sync.dma_start(kfi_s[:], kfidx[:])

            for th in range(2):
                toks = slice(th * TH, (th + 1) * TH)
                rq = [hpool.tile([128, TH], BF16, tag=f"rq{h}", name=f"rq{h}") for h in range(2)]
                for h in range(2):
                    nc.sync.dma_start_transpose(
                        rq[h][:], rT_d2[toks, 128 * h:128 * (h + 1)])
                kfT = hpool.tile([128, TH // 512, 2, 512], BF16, tag="kft")
                for gq in range(TH // 512):
                    nc.gpsimd.dma_gather(
                        kfT[:, gq, :, :], KFtabD[:],
                        kfi_s[:, th * (TH // 16) + 32 * gq:
                              th * (TH // 16) + 32 * (gq + 1)],
                        512, 512, DK, transpose=True)
                fq = hpool.tile([128, 2, TH], BF16, tag="fq")
                for m in range(2):
                    for n in range(TH // 512):
                        nsl = slice(512 * n, 512 * (n + 1))
                        ps = hps.tile([128, 512], F32, tag="psg")
                        nc.tensor.matmul(out=ps[:], lhsT=f1[0][:, 128 * m:128 * (m + 1)],
                                         rhs=rq[0][:, nsl], start=True, stop=False)
                        nc.tensor.matmul(out=ps[:], lhsT=f1[1][:, 128 * m:128 * (m + 1)],
                                         rhs=rq[1][:, nsl], start=False, stop=True)
                        nc.vector.tensor_tensor(out=fq[:, m, nsl], in0=ps[:],
                                                in1=kfT[:, n, m, :], op=ALU.add)
                    nc.scalar.activation(out=fq[:, m, :], in_=fq[:, m, :],
                                         func=ACTF.Tanh, bias=fb_s[:, m:m + 1])
                for n in range(TH // 512):
                    nsl = slice(512 * n, 512 * (n + 1))
                    ps2 = hps2.tile([1, 512], F32, tag="psp")
                    nc.tensor.matmul(out=ps2[:], lhsT=pw_s[:, 0:1],
                                     rhs=fq[:, 0, nsl], start=True, stop=False)
                    nc.tensor.matmul(out=ps2[:], lhsT=pw_s[:, 1:2],
                                     rhs=fq[:, 1, nsl], start=False, stop=True)
                    nc.scalar.activation(out=pred_sb[:, th * TH + 512 * n:
                                                     th * TH + 512 * (n + 1)],
                                         in_=ps2[:], func=ACTF.Sigmoid,
                                         bias=pb_s[:])
            nc.sync.dma_start(pred[:], pred_sb[:])

    nc.finalize()
    return nc


def _host_shared(k_emb, v_emb, Mk, Mv0, e_W, e_b, a_W, a_b, f_W, f_b, p_W, p_b):
    pad_k = np.zeros((NIT, DK), np.float32)
    pad_k[:NUM_ITEM] = k_emb
    pad_v = np.zeros((NX, DK), np.float32)
    pad_v[:2 * NUM_ITEM] = v_emb
    fbcol = np.zeros((128, 2), np.float32)
    fbcol[:, 0] = f_b[:128]
    fbcol[:, 1] = f_b[128:]
    pwcol = np.zeros((128, 2), np.float32)
    pwcol[:, 0] = p_W[0, :128]
    pwcol[:, 1] = p_W[0, 128:]
    # m0[p, k2*128+v] = Mv0[v, kq*64+k2] with p = kq*32 + b
    kq = np.arange(P) // BL
    k2i, vi = np.meshgrid(np.arange(K2), np.arange(DV), indexing="ij")
    m0 = Mv0.T[(kq[:, None, None] * K2 + k2i[None]), vi[None]].reshape(P, SEG)
    # wxidx: item(x) for x in [0, NX)
    items = np.zeros(NX, np.int64)
    items[:2 * NUM_ITEM] = np.arange(2 * NUM_ITEM) % NUM_ITEM
    return {
        "kT": np.ascontiguousarray(pad_k.T).astype(BF),
        "vT": np.ascontiguousarray(pad_v.T).astype(BF),
        "MkT": np.ascontiguousarray(Mk.T).astype(BF),
        "eaWT": np.ascontiguousarray(np.concatenate([e_W.T, a_W.T], axis=1)).astype(BF),
        "fW2T": np.ascontiguousarray(f_W[:, DK:].T).astype(BF),
        "fW1T": np.ascontiguousarray(f_W[:, :DK].T).astype(BF),
        "onesf": np.ones((1, 128), np.float32),
        "eab": np.concatenate([e_b, a_b])[None, :].astype(np.float32),
        "fbcol": fbcol,
        "pwcol": pwcol.astype(BF),
        "pbt": np.array([[float(p_b[0])]], np.float32),
        "m0": m0.astype(BF),
        "wxidx": _wrap16(items),
    }


def _host_core(item_c, x_c):
    """Per-core index tensors. item_c/x_c: [BL, T] int64. p = kq*32 + b."""
    b = np.arange(P) % BL
    kq = np.arange(P) // BL
    xp = x_c[b[None, :], np.arange(T)[:, None]]       # [T, P]
    rows = xp * KSUB + kq[None, :]                     # [T, P]
    cidx = np.zeros((P, NCH, 16), np.int16)
    for ch in range(NCH):
        cidx[:, ch, :] = _wrap16(rows[C * ch:C * (ch + 1), :].reshape(-1))
    return {
        "cidx": cidx.reshape(P, NCH * 16),
        "kfidx": _wrap16(item_c.reshape(-1)),
    }


def kernel(**inputs):
    inputs = {k: np.asarray(v) for k, v in inputs.items()}
    item = inputs["item_seq"].astype(np.int64)
    corr = inputs["correct_seq"].astype(np.int64)
    x = item + NUM_ITEM * corr

    if "nc" not in _cache:
        _cache["nc"] = build_program()
    nc = _cache["nc"]

    shared = _host_shared(
        inputs["k_emb"].astype(np.float32), inputs["v_emb"].astype(np.float32),
        inputs["Mk"].astype(np.float32), inputs["Mv0"].astype(np.float32),
        inputs["e_W"].astype(np.float32), inputs["e_b"].astype(np.float32),
        inputs["a_W"].astype(np.float32), inputs["a_b"].astype(np.float32),
        inputs["f_W"].astype(np.float32), inputs["f_b"].astype(np.float32),
        inputs["p_W"].astype(np.float32), inputs["p_b"].astype(np.float32))

    in_maps = []
    for c in range(NC):
        sl = slice(c * BL, (c + 1) * BL)
        m = dict(shared)
        m.update(_host_core(item[sl], x[sl]))
        in_maps.append(m)

    tdir = os.environ.get("BASS_KERNEL_TRACE_DIR")
    res = run_bass_kernel_spmd(nc, in_maps, core_ids=list(range(NC)),
                               tmpdir=tdir if tdir else None)
    global LAST_RESULT
    LAST_RESULT = res

    out = np.zeros((B, T), np.float32)
    for c in range(NC):
        pr = res.results[c]["pred"].reshape(BL, T)   # tok = b*T + t
        out[c * BL:(c + 1) * BL, :] = pr
    return out


if __name__ == "__main__":
    import time
    rng = np.random.default_rng(0)
    s = 0.05
    ins = {
        "item_seq": rng.integers(0, NUM_ITEM, (B, T)),
        "correct_seq": rng.integers(0, 2, (B, T)),
        "k_emb": (rng.standard_normal((NUM_ITEM, DK)) * s).astype(np.float32),
        "v_emb": (rng.standard_normal((2 * NUM_ITEM, DK)) * s).astype(np.float32),
        "Mk": (rng.standard_normal((DV, DK)) * s).astype(np.float32),
        "Mv0": (rng.standard_normal((DV, DK)) * s).astype(np.float32),
        "e_W": (rng.standard_normal((DK, DK)) * s).astype(np.float32),
        "e_b": np.zeros(DK, np.float32),
        "a_W": (rng.standard_normal((DK, DK)) * s).astype(np.float32),
        "a_b": np.zeros(DK, np.float32),
        "f_W": (rng.standard_normal((DK, 2 * DK)) * s).astype(np.float32),
        "f_b": np.zeros(DK, np.float32),
        "p_W": (rng.standard_normal((1, DK)) * s).astype(np.float32),
        "p_b": np.zeros(1, np.float32),
    }
    t0 = time.time()
    out = kernel(**ins)
    print("kernel wall:", time.time() - t0)

    k = ins["k_emb"][ins["item_seq"]]
    v = ins["v_emb"][ins["item_seq"] + NUM_ITEM * ins["correct_seq"]]
    logits = k @ ins["Mk"].T
    w = np.exp(logits - logits.max(-1, keepdims=True))
    w /= w.sum(-1, keepdims=True)
    e = 1 / (1 + np.exp(-(v @ ins["e_W"].T + ins["e_b"])))
    a = np.tanh(v @ ins["a_W"].T + ins["a_b"])
    M = np.broadcast_to(ins["Mv0"][None], (B, DV, DK)).copy()
    reads = np.zeros((B, T, DK), np.float32)
    for t in range(T):
        reads[:, t] = np.einsum("bv,bvk->bk", w[:, t], M)
        M = M * (1 - w[:, t][:, :, None] * e[:, t][:, None, :]) \
            + w[:, t][:, :, None] * a[:, t][:, None, :]
    f = np.tanh(np.concatenate([reads, k], -1) @ ins["f_W"].T + ins["f_b"])
    ref = 1 / (1 + np.exp(-(f @ ins["p_W"].T + ins["p_b"])))[:, :, 0]
    err = np.abs(out - ref)
    print("max abs err:", err.max(), " rel:", err.max() / np.abs(ref).max())
